# revision 9
# baseline (speedup 1.0000x reference)
"""Trainium2 Bass kernel for nn_AttentionBranch (sparse GQA attention + RoPE).

Problem (hardcoded): B=1, S=2176, 32 q heads, 8 kv heads, head_dim=128,
mask = causal & (sliding-window-256 | kv < 128 meta prefix), fp32 io.

Sharding: 8 cores; core c owns q heads [4c, 4c+4) and kv head c (GQA group).

Per-core dataflow (SPMD, one Bass program):
  - RoPE applied on-device to Q^T / K^T (d-major layout) via 3 DVE ops each,
    using host-precomputed cos / sign-folded-sin tables and half-swapped
    copies of q/k.
  - Block-sparse attention over 128-row q blocks: kv blocks {0, i-2, i-1, i}.
    Scores are computed transposed (kv on partitions): one matmul per
    kv-strip, exp on ScalarE (scale folded in; no max subtraction - scores
    are O(5) for randn inputs), triangular masks via bf16 0/1 multiplies,
    row sums via ones-matmul broadcast into PSUM, PV via V-stationary
    matmuls, final normalize with reciprocal_approx_fast + tensor_mul.
  - Output written d-major [head, dv, q]; host transposes back.
"""

import math
import os
from functools import lru_cache

import numpy as np
import ml_dtypes

S = 2176
D = 128
NB = S // 128  # 17 q/kv blocks
HQ_PER_CORE = 4
N_CORES = 8
WINDOW = 256
META = 128
ROPE_BASE = 10000.0
SCALE = 1.0 / math.sqrt(D)

BF16 = ml_dtypes.bfloat16
LAST_RESULT = None

# q-block ranges for the two halves of a head (psum capacity): [lo, hi] incl.
HALVES = [(0, 7), (8, 16)]


def _strips_for_half(b0, b1):
    """Work list for q-blocks [b0, b1]. Each strip is one kv-block (or meta
    chunk) x a contiguous span of q columns.

    Returns list of dicts:
      kvblk: kv block index j (lhsT = ropeK[:, j*128:(j+1)*128])
      qlo, qhi: global q column range [qlo, qhi)
      meta: True if this is a meta chunk (start=True accumulation)
      diag_u / i2_u: strip-relative column offset of the causal-diag /
        window-tail masked 128-col group, or None.
    """
    strips = []
    lo_col = b0 * 128
    hi_col = (b1 + 1) * 128
    # meta chunks: kv block 0, dense except causal diag for q-block 0,
    # 512-aligned relative to the half so each chunk fills one PSUM bank.
    col = lo_col
    while col < hi_col:
        span = min(512, hi_col - col)
        strips.append(
            dict(
                kvblk=0,
                qlo=col,
                qhi=col + span,
                meta=True,
                diag_u=0 if col == 0 else None,
                i2_u=None,
            )
        )
        col += span
    # window strips: kv block j covers q blocks {j, j+1, j+2} (j >= 1).
    for j in range(1, NB):
        i0 = max(j, b0)
        i1 = min(j + 2, b1)
        if i0 > i1:
            continue
        strips.append(
            dict(
                kvblk=j,
                qlo=i0 * 128,
                qhi=(i1 + 1) * 128,
                meta=False,
                diag_u=0 if i0 == j else None,
                i2_u=(i1 - i0) * 128 if i1 == j + 2 else None,
            )
        )
    return strips


@lru_cache(maxsize=1)
def _build_program():
    import concourse.mybir as mybir
    import concourse.tile as tile
    from concourse import bacc

    bf = mybir.dt.bfloat16
    f32 = mybir.dt.float32
    EXP = mybir.ActivationFunctionType.Exp

    nc = bacc.Bacc(None)

    qt_d = nc.declare_dram_parameter("qt", [HQ_PER_CORE, 2, D, S], bf, isOutput=False)
    kt_d = nc.declare_dram_parameter("kt", [2, D, S], bf, isOutput=False)
    v_d = nc.declare_dram_parameter("v", [D, NB, D], bf, isOutput=False)
    cs_d = nc.declare_dram_parameter("cs", [2, D, S], bf, isOutput=False)
    msk_d = nc.declare_dram_parameter("msk", [D, 2, 128], bf, isOutput=False)
    out_d = nc.declare_dram_parameter("out", [HQ_PER_CORE, D, S], f32, isOutput=True)

    with tile.TileContext(nc) as tc:
        with (
            tc.tile_pool(name="persist", bufs=1) as persist,
            tc.tile_pool(name="ropeq", bufs=2) as ropeq_pool,
            tc.tile_pool(name="tmp", bufs=2) as tmp_pool,
            tc.tile_pool(name="probs", bufs=4) as probs_pool,
            tc.tile_pool(name="norm", bufs=2) as norm_pool,
            tc.tile_pool(name="osb", bufs=2) as osb_pool,
            tc.tile_pool(name="strip", bufs=2, space="PSUM") as strip_psum,
            tc.tile_pool(name="acc", bufs=1, space="PSUM") as acc_psum,
        ):
            qt = persist.tile([D, HQ_PER_CORE, 2, S], bf)
            kt = persist.tile([D, 2, S], bf)
            vt = persist.tile([D, NB, D], bf)
            cs = persist.tile([D, 2, S], bf)
            msk = persist.tile([D, 2, 128], bf)
            ones = persist.tile([D, 128], bf)
            ropek = persist.tile([D, S], bf)

            nc.sync.dma_start(out=qt, in_=qt_d.rearrange("h s d t -> d h s t"))
            nc.sync.dma_start(out=kt, in_=kt_d.rearrange("s d t -> d s t"))
            nc.sync.dma_start(out=vt, in_=v_d[:])
            nc.sync.dma_start(out=cs, in_=cs_d.rearrange("s d t -> d s t"))
            nc.sync.dma_start(out=msk, in_=msk_d[:])
            nc.vector.memset(ones, 1.0)

            def rope(dst, x, xswap):
                t = tmp_pool.tile([D, S], bf, tag="ropetmp")
                nc.vector.tensor_mul(dst, x, cs[:, 0])
                nc.vector.tensor_mul(t, xswap, cs[:, 1])
                nc.vector.tensor_add(dst, dst, t)

            rope(ropek, kt[:, 0], kt[:, 1])

            for h in range(HQ_PER_CORE):
                ropeq = ropeq_pool.tile([D, S], bf, tag="ropeq")
                rope(ropeq, qt[:, h, 0], qt[:, h, 1])

                for b0, b1 in HALVES:
                    hw = (b1 - b0 + 1) * 128
                    zb = acc_psum.tile([D, 1152], f32, tag="zb")
                    ot = acc_psum.tile([D, 1152], f32, tag="ot")

                    # Pre-plan the PSUM-bank segments of every strip so each
                    # 2KB zero-region sees exactly one start=True (first
                    # writer, a meta chunk) and one stop=True (last writer).
                    strips = _strips_for_half(b0, b1)
                    seglists = []  # aligned with strips: list of (lo, hi)
                    last_for_bank = {}
                    for si, st in enumerate(strips):
                        rel = st["qlo"] - b0 * 128
                        span = st["qhi"] - st["qlo"]
                        segs = []
                        seg = rel
                        while seg < rel + span:
                            seg_end = min(rel + span, (seg // 512 + 1) * 512)
                            segs.append((seg, seg_end))
                            last_for_bank[seg // 512] = (si, seg)
                            seg = seg_end
                        seglists.append(segs)
                    last_set = set(last_for_bank.values())

                    for si, st in enumerate(strips):
                        j = st["kvblk"]
                        span = st["qhi"] - st["qlo"]
                        rel = st["qlo"] - b0 * 128

                        sp = strip_psum.tile([D, 512], f32, tag="sp")
                        nc.tensor.matmul(
                            sp[:, :span],
                            lhsT=ropek[:, j * 128 : (j + 1) * 128],
                            rhs=ropeq[:, st["qlo"] : st["qhi"]],
                            start=True,
                            stop=True,
                        )
                        pb = probs_pool.tile([D, 512], bf, tag="pb")
                        nc.scalar.activation(pb[:, :span], sp[:, :span], EXP, scale=SCALE)
                        if st["diag_u"] is not None:
                            u = st["diag_u"]
                            nc.vector.tensor_mul(
                                pb[:, u : u + 128], pb[:, u : u + 128], msk[:, 0]
                            )
                        if st["i2_u"] is not None:
                            u = st["i2_u"]
                            nc.vector.tensor_mul(
                                pb[:, u : u + 128], pb[:, u : u + 128], msk[:, 1]
                            )
                        for seg, seg_end in seglists[si]:
                            w0 = seg - rel
                            w1 = seg_end - rel
                            stop = (si, seg) in last_set
                            nc.tensor.matmul(
                                zb[:, seg:seg_end],
                                lhsT=ones,
                                rhs=pb[:, w0:w1],
                                start=st["meta"],
                                stop=stop,
                            )
                            nc.tensor.matmul(
                                ot[:, seg:seg_end],
                                lhsT=vt[:, j],
                                rhs=pb[:, w0:w1],
                                start=st["meta"],
                                stop=stop,
                            )

                    rz = norm_pool.tile([D, 1152], f32, tag="rz")
                    nc.vector.reciprocal_approx_fast(rz[:, :hw], zb[:, :hw])
                    osb = osb_pool.tile([D, 1152], f32, tag="osb")
                    nc.vector.tensor_mul(osb[:, :hw], ot[:, :hw], rz[:, :hw])
                    nc.sync.dma_start(
                        out=out_d[h, :, b0 * 128 : (b1 + 1) * 128], in_=osb[:, :hw]
                    )

    nc.finalize()
    return nc


@lru_cache(maxsize=1)
def _rope_tables():
    inv_freq = 1.0 / (ROPE_BASE ** (np.arange(0, D, 2, dtype=np.float64) / D))
    pos = np.arange(S, dtype=np.float64)
    freqs = pos[:, None] * inv_freq[None, :]  # [S, 64]
    emb = np.concatenate([freqs, freqs], axis=-1)  # [S, D]
    # match the f32 reference: compute cos/sin at f32 granularity
    cosT = np.cos(emb.astype(np.float32)).T.astype(np.float32)  # [D, S]
    sinT = np.sin(emb.astype(np.float32)).T.astype(np.float32)
    sinTpm = np.concatenate([-sinT[:64], sinT[64:]], axis=0)
    return cosT, sinTpm


def _mask_tiles():
    c = np.arange(128)[:, None]
    u = np.arange(128)[None, :]
    diag_keep = (u >= c).astype(np.float32)  # causal diag block
    i2_keep = (u <= c).astype(np.float32)  # window tail block
    return np.stack([diag_keep, i2_keep], axis=1)  # [128, 2, 128]


def _swap_halves(xT):
    return np.concatenate([xT[64:], xT[:64]], axis=0)


def _install_ntff_shim():
    """Provide antenv.axon_hooks (NTFF profile hook) if the image lacks it,
    so run_bass_kernel_spmd(trace=True) can capture HW profiles via the
    axon PJRT .so. Silently no-ops if unavailable."""
    import sys
    import types

    try:
        from antenv.axon_hooks import get_axon_ntff_profile_hook  # noqa: F401

        return
    except ImportError:
        pass
    try:
        import contextlib
        import ctypes

        lib = ctypes.CDLL("/opt/axon/libaxon_pjrt.so")
        if not hasattr(lib, "axon_start_nrt_profile"):
            return
        lib.axon_start_nrt_profile.argtypes = [
            ctypes.POINTER(ctypes.c_int64),
            ctypes.c_size_t,
        ]
        lib.axon_start_nrt_profile.restype = ctypes.c_int64
        lib.axon_stop_nrt_profile.argtypes = [ctypes.c_char_p]
        lib.axon_stop_nrt_profile.restype = ctypes.c_int64

        @contextlib.contextmanager
        def _hook(output_dir, device_ids):
            import jax

            jax.devices()
            if device_ids:
                ids = (ctypes.c_int64 * len(device_ids))(*device_ids)
                rc = lib.axon_start_nrt_profile(ids, len(device_ids))
            else:
                rc = lib.axon_start_nrt_profile(None, 0)
            if rc != 0:
                raise RuntimeError(f"axon_start_nrt_profile rc={rc}")
            try:
                yield
            finally:
                n = lib.axon_stop_nrt_profile(str(output_dir).encode())
                print(f"ntff profile: {n} file(s) -> {output_dir}", file=sys.stderr)

        mod = types.ModuleType("antenv.axon_hooks")
        mod._hook = _hook
        mod.get_axon_ntff_profile_hook = lambda: _hook
        mod.set_axon_ntff_profile_hook = lambda h: setattr(mod, "_hook", h)
        import antenv

        antenv.axon_hooks = mod
        sys.modules["antenv.axon_hooks"] = mod
    except Exception:
        pass


def kernel(query_states, key_states, value_states):
    from concourse.bass_utils import run_bass_kernel_spmd

    _install_ntff_shim()

    nc = _build_program()

    q = np.asarray(query_states)[0]  # [S, 4096]
    k = np.asarray(key_states)[0]  # [S, 1024]
    v = np.asarray(value_states)[0]  # [S, 1024]

    cosT, sinTpm = _rope_tables()
    cs = np.stack([cosT, sinTpm], axis=0).astype(BF16)  # [2, D, S]
    msk = _mask_tiles().astype(BF16)

    in_maps = []
    for c in range(N_CORES):
        qt = np.empty((HQ_PER_CORE, 2, D, S), dtype=BF16)
        for hh in range(HQ_PER_CORE):
            h = 4 * c + hh
            qh = np.ascontiguousarray(q[:, h * D : (h + 1) * D].T)  # [D, S]
            qt[hh, 0] = qh.astype(BF16)
            qt[hh, 1] = _swap_halves(qh).astype(BF16)
        kh = np.ascontiguousarray(k[:, c * D : (c + 1) * D].T)
        kt = np.stack([kh, _swap_halves(kh)], axis=0).astype(BF16)
        vh = v[:, c * D : (c + 1) * D]  # [S, D]
        vts = np.ascontiguousarray(
            vh.reshape(NB, 128, D).transpose(1, 0, 2)
        ).astype(BF16)  # [kv_local, j, dv]
        in_maps.append({"qt": qt, "kt": kt, "v": vts, "cs": cs, "msk": msk})

    res = run_bass_kernel_spmd(nc, in_maps, core_ids=list(range(N_CORES)))
    global LAST_RESULT
    LAST_RESULT = res

    out = np.empty((S, 32, D), dtype=np.float32)
    for c in range(N_CORES):
        o = res.results[c]["out"]  # [4, D, S] f32
        out[:, 4 * c : 4 * c + 4, :] = o.transpose(2, 0, 1)
    return out.reshape(1, S, 32 * D)


# revision 13
# speedup vs baseline: 1.0308x; 1.0308x over previous
"""Trainium2 Bass kernel for nn_AttentionBranch (sparse GQA attention + RoPE).

Problem (hardcoded): B=1, S=2176, 32 q heads, 8 kv heads, head_dim=128,
mask = causal & (sliding-window-256 | kv < 128 meta prefix), fp32 io.

Sharding: 8 cores; core c owns q heads [4c, 4c+4) and kv head c (GQA group).

Per-core dataflow (SPMD, one Bass program):
  - RoPE applied on-device to Q^T / K^T (d-major layout) via 3 DVE ops each,
    using host-precomputed cos / sign-folded-sin tables and half-swapped
    copies of q/k.
  - Block-sparse attention over 128-row q blocks: kv blocks {0, i-2, i-1, i}.
    Scores are computed transposed (kv on partitions): one matmul per
    kv-strip, exp on ScalarE (scale folded in; no max subtraction - scores
    are O(5) for randn inputs), triangular masks via bf16 0/1 multiplies,
    row sums via ones-matmul broadcast into PSUM, PV via V-stationary
    matmuls, final normalize with reciprocal_approx_fast + tensor_mul.
  - Output written d-major [head, dv, q]; host transposes back.
"""

import math
import os
from functools import lru_cache

import numpy as np
import ml_dtypes

S = 2176
D = 128
NB = S // 128  # 17 q/kv blocks
HQ_PER_CORE = 4
N_CORES = 8
WINDOW = 256
META = 128
ROPE_BASE = 10000.0
SCALE = 1.0 / math.sqrt(D)

BF16 = ml_dtypes.bfloat16
LAST_RESULT = None

# q-block ranges of the per-head processing pieces (PSUM capacity: each
# piece's Z/out accumulators are <=2 banks; strip pairs use the other 4).
PIECES = [(0, 4), (5, 8), (9, 12), (13, 16)]


def _strips_for_piece(b0, b1):
    """Work list for q-blocks [b0, b1]. Each strip is one kv-block (or meta
    chunk) x a contiguous span of q columns.

    Returns list of dicts:
      kvblk: kv block index j (lhsT = ropeK[:, j*128:(j+1)*128])
      qlo, qhi: global q column range [qlo, qhi)
      meta: True if this is a meta chunk (start=True accumulation)
      diag_u / i2_u: strip-relative column offset of the causal-diag /
        window-tail masked 128-col group, or None.
    """
    strips = []
    lo_col = b0 * 128
    hi_col = (b1 + 1) * 128
    # meta chunks: kv block 0, dense except causal diag for q-block 0,
    # 512-aligned relative to the piece so each chunk fills one PSUM bank.
    col = lo_col
    while col < hi_col:
        span = min(512, hi_col - col)
        strips.append(
            dict(
                kvblk=0,
                qlo=col,
                qhi=col + span,
                meta=True,
                diag_u=0 if col == 0 else None,
                i2_u=None,
            )
        )
        col += span
    # window strips: kv block j covers q blocks {j, j+1, j+2} (j >= 1).
    for j in range(1, NB):
        i0 = max(j, b0)
        i1 = min(j + 2, b1)
        if i0 > i1:
            continue
        strips.append(
            dict(
                kvblk=j,
                qlo=i0 * 128,
                qhi=(i1 + 1) * 128,
                meta=False,
                diag_u=0 if i0 == j else None,
                i2_u=(i1 - i0) * 128 if i1 == j + 2 else None,
            )
        )
    return strips


def _pair_strips(strips):
    """Pack strips two-per-PSUM-pair-tile ([128, 1024] = 2 banks). Each strip
    gets an `off` column inside the tile such that its matmul output stays
    within one 512-col bank. Returns list of pairs (1-2 strips each)."""
    pairs = []
    i = 0
    while i < len(strips):
        group = [strips[i]]
        strips[i]["off"] = 0
        span0 = strips[i]["qhi"] - strips[i]["qlo"]
        if i + 1 < len(strips):
            s1 = strips[i + 1]
            span1 = s1["qhi"] - s1["qlo"]
            s1["off"] = span0 if span0 + span1 <= 512 or span0 == 512 else 512
            group.append(s1)
            i += 2
        else:
            i += 1
        pairs.append(group)
    return pairs


@lru_cache(maxsize=1)
def _build_program():
    import concourse.bass as bass
    import concourse.mybir as mybir
    import concourse.tile as tile
    from concourse import bacc

    bf = mybir.dt.bfloat16
    f32 = mybir.dt.float32
    EXP = mybir.ActivationFunctionType.Exp

    nc = bacc.Bacc(None)

    qt_d = nc.declare_dram_parameter("qt", [HQ_PER_CORE, 2, D, S], bf, isOutput=False)
    kt_d = nc.declare_dram_parameter("kt", [2, D, S], bf, isOutput=False)
    v_d = nc.declare_dram_parameter("v", [D, NB, D], bf, isOutput=False)
    cs_d = nc.declare_dram_parameter("cs", [2, D, S], bf, isOutput=False)
    msk_d = nc.declare_dram_parameter("msk", [D, 2, 128], bf, isOutput=False)
    out_d = nc.declare_dram_parameter("out", [HQ_PER_CORE, D, S], f32, isOutput=True)

    with tile.TileContext(nc) as tc:
        with (
            tc.tile_pool(name="persist", bufs=1) as persist,
            tc.tile_pool(name="probs", bufs=6) as probs_pool,
            tc.tile_pool(name="norm", bufs=3) as norm_pool,
            tc.tile_pool(name="osb", bufs=3) as osb_pool,
            tc.tile_pool(name="strip", bufs=2, space="PSUM") as strip_psum,
            tc.tile_pool(name="acc", bufs=1, space="PSUM") as acc_psum,
        ):
            qt = persist.tile([D, HQ_PER_CORE, 2, S], bf)
            kt = persist.tile([D, 2, S], bf)
            vt = persist.tile([D, NB, D], bf)
            cs = persist.tile([D, 2, S], bf)
            msk = persist.tile([D, 2, 128], bf)
            ones = persist.tile([D, 128], bf)
            ropek = persist.tile([D, S], bf)
            ropeq = persist.tile([D, HQ_PER_CORE, S], bf)
            ropet = persist.tile([D, HQ_PER_CORE, S], bf)

            nc.sync.dma_start(out=qt, in_=qt_d.rearrange("h s d t -> d h s t"))
            nc.sync.dma_start(out=kt, in_=kt_d.rearrange("s d t -> d s t"))
            nc.sync.dma_start(out=vt, in_=v_d[:])
            nc.sync.dma_start(out=cs, in_=cs_d.rearrange("s d t -> d s t"))
            nc.sync.dma_start(out=msk, in_=msk_d[:])
            nc.vector.memset(ones, 1.0)

            # RoPE: K first (QK critical path), then all 4 Q heads in three
            # wide ops (cos/sin broadcast over the head dim via step-0 APs).
            nc.vector.tensor_mul(ropek, kt[:, 0], cs[:, 0])
            nc.vector.tensor_mul(ropet[:, 0], kt[:, 1], cs[:, 1])
            nc.vector.tensor_add(ropek, ropek, ropet[:, 0])
            cos_b = cs[:, 0:1, :].broadcast_to([D, HQ_PER_CORE, S])
            sin_b = cs[:, 1:2, :].broadcast_to([D, HQ_PER_CORE, S])
            nc.vector.tensor_mul(ropeq, qt[:, :, 0], cos_b)
            nc.vector.tensor_mul(ropet, qt[:, :, 1], sin_b)
            nc.vector.tensor_add(ropeq, ropeq, ropet)

            for h in range(HQ_PER_CORE):
                for b0, b1 in PIECES:
                    pw = (b1 - b0 + 1) * 128
                    zb = acc_psum.tile([D, 640], f32, tag="zb")
                    ot = acc_psum.tile([D, 640], f32, tag="ot")

                    # Plan strips, PSUM-bank segments, and per-bank last
                    # writers so every 2KB zero-region sees one start=True
                    # (its meta chunk) and one stop=True (last writer).
                    strips = _strips_for_piece(b0, b1)
                    seglists = []
                    last_for_bank = {}
                    for si, st in enumerate(strips):
                        rel = st["qlo"] - b0 * 128
                        span = st["qhi"] - st["qlo"]
                        segs = []
                        seg = rel
                        while seg < rel + span:
                            seg_end = min(rel + span, (seg // 512 + 1) * 512)
                            segs.append((seg, seg_end))
                            last_for_bank[seg // 512] = (si, seg)
                            seg = seg_end
                        seglists.append(segs)
                    last_set = set(last_for_bank.values())
                    pairs = _pair_strips(strips)

                    si = 0
                    for pair in pairs:
                        sp = strip_psum.tile([D, 1024], f32, tag="sp")
                        pb = probs_pool.tile([D, 1024], bf, tag="pb")
                        for st in pair:
                            span = st["qhi"] - st["qlo"]
                            o = st["off"]
                            nc.tensor.matmul(
                                sp[:, o : o + span],
                                lhsT=ropek[
                                    :, st["kvblk"] * 128 : (st["kvblk"] + 1) * 128
                                ],
                                rhs=ropeq[:, h, st["qlo"] : st["qhi"]],
                                start=True,
                                stop=True,
                            )
                        # exp over the pair: one ACT call when the layout
                        # allows (contiguous, or equal-span grouped AP).
                        if len(pair) == 1:
                            st = pair[0]
                            span = st["qhi"] - st["qlo"]
                            nc.scalar.activation(
                                pb[:, :span], sp[:, :span], EXP, scale=SCALE
                            )
                        else:
                            s0, s1 = pair
                            sp0 = s0["qhi"] - s0["qlo"]
                            sp1 = s1["qhi"] - s1["qlo"]
                            if s1["off"] == sp0:
                                nc.scalar.activation(
                                    pb[:, : sp0 + sp1],
                                    sp[:, : sp0 + sp1],
                                    EXP,
                                    scale=SCALE,
                                )
                            elif sp0 == sp1:
                                gap = s1["off"]
                                src = sp.rearrange("d (g t) -> d g t", g=2)[
                                    :, :, :sp0
                                ]
                                dst = pb.rearrange("d (g t) -> d g t", g=2)[
                                    :, :, :sp0
                                ]
                                assert gap == 512
                                nc.scalar.activation(dst, src, EXP, scale=SCALE)
                            else:
                                nc.scalar.activation(
                                    pb[:, :sp0], sp[:, :sp0], EXP, scale=SCALE
                                )
                                nc.scalar.activation(
                                    pb[:, s1["off"] : s1["off"] + sp1],
                                    sp[:, s1["off"] : s1["off"] + sp1],
                                    EXP,
                                    scale=SCALE,
                                )
                        # masks: one DVE op per strip (grouped AP when a strip
                        # has both a diag and a window-tail masked block).
                        for st in pair:
                            o = st["off"]
                            du, iu = st["diag_u"], st["i2_u"]
                            if du is not None and iu is not None:
                                stride = iu - du
                                src = bass.AP(
                                    tensor=pb.tensor,
                                    offset=pb[:, o + du : o + du + 1].offset,
                                    ap=[pb.ap[0], [stride, 2], [1, 128]],
                                )
                                nc.vector.tensor_mul(src, src, msk)
                            elif du is not None:
                                nc.vector.tensor_mul(
                                    pb[:, o + du : o + du + 128],
                                    pb[:, o + du : o + du + 128],
                                    msk[:, 0],
                                )
                            elif iu is not None:
                                nc.vector.tensor_mul(
                                    pb[:, o + iu : o + iu + 128],
                                    pb[:, o + iu : o + iu + 128],
                                    msk[:, 1],
                                )
                        # row-sum (partition-broadcast via ones) + PV matmuls
                        for st in pair:
                            rel = st["qlo"] - b0 * 128
                            o = st["off"]
                            for seg, seg_end in seglists[si]:
                                w0 = o + seg - rel
                                w1 = o + seg_end - rel
                                stop = (si, seg) in last_set
                                nc.tensor.matmul(
                                    zb[:, seg:seg_end],
                                    lhsT=ones,
                                    rhs=pb[:, w0:w1],
                                    start=st["meta"],
                                    stop=stop,
                                )
                                nc.tensor.matmul(
                                    ot[:, seg:seg_end],
                                    lhsT=vt[:, st["kvblk"]],
                                    rhs=pb[:, w0:w1],
                                    start=st["meta"],
                                    stop=stop,
                                )
                            si += 1

                    rz = norm_pool.tile([D, 640], f32, tag="rz")
                    nc.vector.reciprocal_approx_fast(rz[:, :pw], zb[:, :pw])
                    osb = osb_pool.tile([D, 640], f32, tag="osb")
                    nc.vector.tensor_mul(osb[:, :pw], ot[:, :pw], rz[:, :pw])
                    nc.sync.dma_start(
                        out=out_d[h, :, b0 * 128 : (b1 + 1) * 128], in_=osb[:, :pw]
                    )

    nc.finalize()
    return nc


@lru_cache(maxsize=1)
def _rope_tables():
    inv_freq = 1.0 / (ROPE_BASE ** (np.arange(0, D, 2, dtype=np.float64) / D))
    pos = np.arange(S, dtype=np.float64)
    freqs = pos[:, None] * inv_freq[None, :]  # [S, 64]
    emb = np.concatenate([freqs, freqs], axis=-1)  # [S, D]
    # match the f32 reference: compute cos/sin at f32 granularity
    cosT = np.cos(emb.astype(np.float32)).T.astype(np.float32)  # [D, S]
    sinT = np.sin(emb.astype(np.float32)).T.astype(np.float32)
    sinTpm = np.concatenate([-sinT[:64], sinT[64:]], axis=0)
    return cosT, sinTpm


def _mask_tiles():
    c = np.arange(128)[:, None]
    u = np.arange(128)[None, :]
    diag_keep = (u >= c).astype(np.float32)  # causal diag block
    i2_keep = (u <= c).astype(np.float32)  # window tail block
    return np.stack([diag_keep, i2_keep], axis=1)  # [128, 2, 128]


def _swap_halves(xT):
    return np.concatenate([xT[64:], xT[:64]], axis=0)


def _install_ntff_shim():
    """Provide antenv.axon_hooks (NTFF profile hook) if the image lacks it,
    so run_bass_kernel_spmd(trace=True) can capture HW profiles via the
    axon PJRT .so. Silently no-ops if unavailable."""
    import sys
    import types

    try:
        from antenv.axon_hooks import get_axon_ntff_profile_hook  # noqa: F401

        return
    except ImportError:
        pass
    try:
        import contextlib
        import ctypes

        lib = ctypes.CDLL("/opt/axon/libaxon_pjrt.so")
        if not hasattr(lib, "axon_start_nrt_profile"):
            return
        lib.axon_start_nrt_profile.argtypes = [
            ctypes.POINTER(ctypes.c_int64),
            ctypes.c_size_t,
        ]
        lib.axon_start_nrt_profile.restype = ctypes.c_int64
        lib.axon_stop_nrt_profile.argtypes = [ctypes.c_char_p]
        lib.axon_stop_nrt_profile.restype = ctypes.c_int64

        @contextlib.contextmanager
        def _hook(output_dir, device_ids):
            import jax

            jax.devices()
            if device_ids:
                ids = (ctypes.c_int64 * len(device_ids))(*device_ids)
                rc = lib.axon_start_nrt_profile(ids, len(device_ids))
            else:
                rc = lib.axon_start_nrt_profile(None, 0)
            if rc != 0:
                raise RuntimeError(f"axon_start_nrt_profile rc={rc}")
            try:
                yield
            finally:
                n = lib.axon_stop_nrt_profile(str(output_dir).encode())
                print(f"ntff profile: {n} file(s) -> {output_dir}", file=sys.stderr)

        mod = types.ModuleType("antenv.axon_hooks")
        mod._hook = _hook
        mod.get_axon_ntff_profile_hook = lambda: _hook
        mod.set_axon_ntff_profile_hook = lambda h: setattr(mod, "_hook", h)
        import antenv

        antenv.axon_hooks = mod
        sys.modules["antenv.axon_hooks"] = mod
    except Exception:
        pass


def kernel(query_states, key_states, value_states):
    from concourse.bass_utils import run_bass_kernel_spmd

    _install_ntff_shim()

    nc = _build_program()

    q = np.asarray(query_states)[0]  # [S, 4096]
    k = np.asarray(key_states)[0]  # [S, 1024]
    v = np.asarray(value_states)[0]  # [S, 1024]

    cosT, sinTpm = _rope_tables()
    cs = np.stack([cosT, sinTpm], axis=0).astype(BF16)  # [2, D, S]
    msk = _mask_tiles().astype(BF16)

    in_maps = []
    for c in range(N_CORES):
        qt = np.empty((HQ_PER_CORE, 2, D, S), dtype=BF16)
        for hh in range(HQ_PER_CORE):
            h = 4 * c + hh
            qh = np.ascontiguousarray(q[:, h * D : (h + 1) * D].T)  # [D, S]
            qt[hh, 0] = qh.astype(BF16)
            qt[hh, 1] = _swap_halves(qh).astype(BF16)
        kh = np.ascontiguousarray(k[:, c * D : (c + 1) * D].T)
        kt = np.stack([kh, _swap_halves(kh)], axis=0).astype(BF16)
        vh = v[:, c * D : (c + 1) * D]  # [S, D]
        vts = np.ascontiguousarray(
            vh.reshape(NB, 128, D).transpose(1, 0, 2)
        ).astype(BF16)  # [kv_local, j, dv]
        in_maps.append({"qt": qt, "kt": kt, "v": vts, "cs": cs, "msk": msk})

    res = run_bass_kernel_spmd(nc, in_maps, core_ids=list(range(N_CORES)))
    global LAST_RESULT
    LAST_RESULT = res

    out = np.empty((S, 32, D), dtype=np.float32)
    for c in range(N_CORES):
        o = res.results[c]["out"]  # [4, D, S] f32
        out[:, 4 * c : 4 * c + 4, :] = o.transpose(2, 0, 1)
    return out.reshape(1, S, 32 * D)


# revision 18
# speedup vs baseline: 1.2259x; 1.1893x over previous
"""Trainium2 Bass kernel for nn_AttentionBranch (sparse GQA attention + RoPE).

Problem (hardcoded): B=1, S=2176, 32 q heads, 8 kv heads, head_dim=128,
mask = causal & (sliding-window-256 | kv < 128 meta prefix), fp32 io.

Sharding: 8 cores; core c owns q heads [4c, 4c+4) and kv head c (GQA group).

Per-core dataflow (SPMD, one Bass program):
  - RoPE applied on-device to Q^T / K^T (d-major layout) via 3 DVE ops each,
    using host-precomputed cos / sign-folded-sin tables and half-swapped
    copies of q/k.
  - Block-sparse attention over 128-row q blocks: kv blocks {0, i-2, i-1, i}.
    Scores are computed transposed (kv on partitions): one matmul per
    kv-strip, exp on ScalarE (scale folded in; no max subtraction - scores
    are O(5) for randn inputs), triangular masks via bf16 0/1 multiplies,
    row sums via ones-matmul broadcast into PSUM, PV via V-stationary
    matmuls, final normalize with reciprocal_approx_fast + tensor_mul.
  - Output written d-major [head, dv, q]; host transposes back.
"""

import math
import os
from functools import lru_cache

import numpy as np
import ml_dtypes

S = 2176
D = 128
NB = S // 128  # 17 q/kv blocks
HQ_PER_CORE = 4
N_CORES = 8
WINDOW = 256
META = 128
ROPE_BASE = 10000.0
SCALE = 1.0 / math.sqrt(D)

BF16 = ml_dtypes.bfloat16
LAST_RESULT = None

# q-block ranges of the per-head processing pieces (PSUM capacity: each
# piece's Z/out accumulators are one bank, double-buffered => 4 banks; the
# strip-pair tiles use the other 4).
PIECES = [(0, 3), (4, 7), (8, 11), (12, 15), (16, 16)]


def _strips_for_piece(b0, b1):
    """Work list for q-blocks [b0, b1]. Each strip is one kv-block (or meta
    chunk) x a contiguous span of q columns.

    Returns list of dicts:
      kvblk: kv block index j (lhsT = ropeK[:, j*128:(j+1)*128])
      qlo, qhi: global q column range [qlo, qhi)
      meta: True if this is a meta chunk (start=True accumulation)
      diag_u / i2_u: strip-relative column offset of the causal-diag /
        window-tail masked 128-col group, or None.
    """
    strips = []
    lo_col = b0 * 128
    hi_col = (b1 + 1) * 128
    # meta chunks: kv block 0, dense except causal diag for q-block 0,
    # 512-aligned relative to the piece so each chunk fills one PSUM bank.
    col = lo_col
    while col < hi_col:
        span = min(512, hi_col - col)
        strips.append(
            dict(
                kvblk=0,
                qlo=col,
                qhi=col + span,
                meta=True,
                diag_u=0 if col == 0 else None,
                i2_u=None,
            )
        )
        col += span
    # window strips: kv block j covers q blocks {j, j+1, j+2} (j >= 1).
    for j in range(1, NB):
        i0 = max(j, b0)
        i1 = min(j + 2, b1)
        if i0 > i1:
            continue
        strips.append(
            dict(
                kvblk=j,
                qlo=i0 * 128,
                qhi=(i1 + 1) * 128,
                meta=False,
                diag_u=0 if i0 == j else None,
                i2_u=(i1 - i0) * 128 if i1 == j + 2 else None,
            )
        )
    return strips


def _pair_strips(strips):
    """Pack strips two-per-PSUM-pair-tile ([128, 1024] = 2 banks). Each strip
    gets an `off` column inside the tile such that its matmul output stays
    within one 512-col bank, and pairs are chosen so the pair's exp can run
    as a single ACT call (contiguous layout, or equal spans at stride 512).
    Returns list of pairs (1-2 strips each)."""

    def span(s):
        return s["qhi"] - s["qlo"]

    rest = sorted(strips, key=lambda s: (not s["meta"], -span(s)))
    pairs = []
    while rest:
        s0 = rest.pop(0)
        sp0 = span(s0)
        s0["off"] = 0
        if not rest:
            pairs.append([s0])
            break
        # prefer a partner that gives a single exp call
        pick = None
        for cand in rest:
            if sp0 == 512 or sp0 + span(cand) <= 512:  # contiguous
                pick = cand
                break
        if pick is None:
            for cand in rest:
                if span(cand) == sp0:  # equal-span grouped AP at stride 512
                    pick = cand
                    break
        if pick is None:
            pick = rest[0]
        rest.remove(pick)
        sp1 = span(pick)
        pick["off"] = sp0 if (sp0 + sp1 <= 512 or sp0 == 512) else 512
        pairs.append([s0, pick])
    return pairs


@lru_cache(maxsize=1)
def _build_program():
    import concourse.bass as bass
    import concourse.mybir as mybir
    import concourse.tile as tile
    from concourse import bacc

    bf = mybir.dt.bfloat16
    f32 = mybir.dt.float32
    EXP = mybir.ActivationFunctionType.Exp

    nc = bacc.Bacc(None)

    qt_d = nc.declare_dram_parameter("qt", [HQ_PER_CORE, 2, D, S], bf, isOutput=False)
    kt_d = nc.declare_dram_parameter("kt", [2, D, S], bf, isOutput=False)
    v_d = nc.declare_dram_parameter("v", [D, NB, D], bf, isOutput=False)
    cs_d = nc.declare_dram_parameter("cs", [2, D, S], bf, isOutput=False)
    msk_d = nc.declare_dram_parameter("msk", [D, 2, 128], bf, isOutput=False)
    out_d = nc.declare_dram_parameter("out", [HQ_PER_CORE, D, S], f32, isOutput=True)

    with tile.TileContext(nc) as tc:
        with (
            tc.tile_pool(name="persist", bufs=1) as persist,
            tc.tile_pool(name="probs", bufs=6) as probs_pool,
            tc.tile_pool(name="norm", bufs=3) as norm_pool,
            tc.tile_pool(name="osb", bufs=3) as osb_pool,
            tc.tile_pool(name="strip", bufs=2, space="PSUM") as strip_psum,
            tc.tile_pool(name="acc", bufs=2, space="PSUM") as acc_psum,
        ):
            qt = persist.tile([D, HQ_PER_CORE, 2, S], bf)
            kt = persist.tile([D, 2, S], bf)
            vt = persist.tile([D, NB, D], bf)
            cs = persist.tile([D, 2, S], bf)
            msk = persist.tile([D, 2, 128], bf)
            ones = persist.tile([D, 128], bf)
            ropek = persist.tile([D, S], bf)
            ropeq = persist.tile([D, HQ_PER_CORE, S], bf)
            ropet = persist.tile([D, HQ_PER_CORE, S], bf)

            nc.sync.dma_start(out=kt, in_=kt_d.rearrange("s d t -> d s t"))
            nc.sync.dma_start(out=cs, in_=cs_d.rearrange("s d t -> d s t"))
            for h in range(HQ_PER_CORE):
                nc.sync.dma_start(
                    out=qt[:, h], in_=qt_d[h].rearrange("s d t -> d s t")
                )
            nc.sync.dma_start(out=vt, in_=v_d[:])
            nc.sync.dma_start(out=msk, in_=msk_d[:])
            nc.vector.memset(ones, 1.0)

            # RoPE: K first (QK critical path), then per-head Q so head 0's
            # attention can start while later heads' inputs still stream in.
            nc.vector.tensor_mul(ropek, kt[:, 0], cs[:, 0])
            nc.vector.tensor_mul(ropet[:, 0], kt[:, 1], cs[:, 1])
            nc.vector.tensor_add(ropek, ropek, ropet[:, 0])
            for h in range(HQ_PER_CORE):
                nc.vector.tensor_mul(ropeq[:, h], qt[:, h, 0], cs[:, 0])
                nc.vector.tensor_mul(ropet[:, h], qt[:, h, 1], cs[:, 1])
                nc.vector.tensor_add(ropeq[:, h], ropeq[:, h], ropet[:, h])

            for h in range(HQ_PER_CORE):
                for b0, b1 in PIECES:
                    pw = (b1 - b0 + 1) * 128
                    zb = acc_psum.tile([D, 512], f32, tag="zb")
                    ot = acc_psum.tile([D, 512], f32, tag="ot")

                    # Plan strips and pair them; then compute PSUM-bank
                    # segments in EMISSION order so every 2KB zero-region
                    # sees one start=True (its meta chunk, always emitted
                    # first) and one stop=True (its last writer).
                    strips = _strips_for_piece(b0, b1)
                    pairs = _pair_strips(strips)
                    last_for_bank = {}
                    for pair in pairs:
                        for st in pair:
                            rel = st["qlo"] - b0 * 128
                            span = st["qhi"] - st["qlo"]
                            segs = []
                            seg = rel
                            while seg < rel + span:
                                seg_end = min(rel + span, (seg // 512 + 1) * 512)
                                segs.append((seg, seg_end))
                                last_for_bank[seg // 512] = (id(st), seg)
                                seg = seg_end
                            st["segs"] = segs
                    last_set = set(last_for_bank.values())

                    for pair in pairs:
                        sp = strip_psum.tile([D, 1024], f32, tag="sp")
                        pb = probs_pool.tile([D, 1024], bf, tag="pb")
                        for st in pair:
                            span = st["qhi"] - st["qlo"]
                            o = st["off"]
                            nc.tensor.matmul(
                                sp[:, o : o + span],
                                lhsT=ropek[
                                    :, st["kvblk"] * 128 : (st["kvblk"] + 1) * 128
                                ],
                                rhs=ropeq[:, h, st["qlo"] : st["qhi"]],
                                start=True,
                                stop=True,
                            )
                        # exp over the pair: one ACT call when the layout
                        # allows (contiguous, or equal-span grouped AP).
                        if len(pair) == 1:
                            st = pair[0]
                            span = st["qhi"] - st["qlo"]
                            nc.scalar.activation(
                                pb[:, :span], sp[:, :span], EXP, scale=SCALE
                            )
                        else:
                            s0, s1 = pair
                            sp0 = s0["qhi"] - s0["qlo"]
                            sp1 = s1["qhi"] - s1["qlo"]
                            if s1["off"] == sp0:
                                nc.scalar.activation(
                                    pb[:, : sp0 + sp1],
                                    sp[:, : sp0 + sp1],
                                    EXP,
                                    scale=SCALE,
                                )
                            elif sp0 == sp1:
                                gap = s1["off"]
                                src = sp.rearrange("d (g t) -> d g t", g=2)[
                                    :, :, :sp0
                                ]
                                dst = pb.rearrange("d (g t) -> d g t", g=2)[
                                    :, :, :sp0
                                ]
                                assert gap == 512
                                nc.scalar.activation(dst, src, EXP, scale=SCALE)
                            else:
                                nc.scalar.activation(
                                    pb[:, :sp0], sp[:, :sp0], EXP, scale=SCALE
                                )
                                nc.scalar.activation(
                                    pb[:, s1["off"] : s1["off"] + sp1],
                                    sp[:, s1["off"] : s1["off"] + sp1],
                                    EXP,
                                    scale=SCALE,
                                )
                        # masks: one DVE op per strip (grouped AP when a strip
                        # has both a diag and a window-tail masked block).
                        for st in pair:
                            o = st["off"]
                            du, iu = st["diag_u"], st["i2_u"]
                            if du is not None and iu is not None:
                                stride = iu - du
                                src = bass.AP(
                                    tensor=pb.tensor,
                                    offset=pb[:, o + du : o + du + 1].offset,
                                    ap=[pb.ap[0], [stride, 2], [1, 128]],
                                )
                                nc.vector.tensor_mul(src, src, msk)
                            elif du is not None:
                                nc.vector.tensor_mul(
                                    pb[:, o + du : o + du + 128],
                                    pb[:, o + du : o + du + 128],
                                    msk[:, 0],
                                )
                            elif iu is not None:
                                nc.vector.tensor_mul(
                                    pb[:, o + iu : o + iu + 128],
                                    pb[:, o + iu : o + iu + 128],
                                    msk[:, 1],
                                )
                        # row-sum (partition-broadcast via ones) + PV matmuls
                        for st in pair:
                            rel = st["qlo"] - b0 * 128
                            o = st["off"]
                            for seg, seg_end in st["segs"]:
                                w0 = o + seg - rel
                                w1 = o + seg_end - rel
                                stop = (id(st), seg) in last_set
                                nc.tensor.matmul(
                                    zb[:, seg:seg_end],
                                    lhsT=ones,
                                    rhs=pb[:, w0:w1],
                                    start=st["meta"],
                                    stop=stop,
                                )
                                nc.tensor.matmul(
                                    ot[:, seg:seg_end],
                                    lhsT=vt[:, st["kvblk"]],
                                    rhs=pb[:, w0:w1],
                                    start=st["meta"],
                                    stop=stop,
                                )

                    rz = norm_pool.tile([D, 512], f32, tag="rz")
                    nc.vector.reciprocal_approx_fast(rz[:, :pw], zb[:, :pw])
                    osb = osb_pool.tile([D, 512], f32, tag="osb")
                    nc.vector.tensor_mul(osb[:, :pw], ot[:, :pw], rz[:, :pw])
                    nc.sync.dma_start(
                        out=out_d[h, :, b0 * 128 : (b1 + 1) * 128], in_=osb[:, :pw]
                    )

    nc.finalize()
    return nc


@lru_cache(maxsize=1)
def _rope_tables():
    inv_freq = 1.0 / (ROPE_BASE ** (np.arange(0, D, 2, dtype=np.float64) / D))
    pos = np.arange(S, dtype=np.float64)
    freqs = pos[:, None] * inv_freq[None, :]  # [S, 64]
    emb = np.concatenate([freqs, freqs], axis=-1)  # [S, D]
    # match the f32 reference: compute cos/sin at f32 granularity
    cosT = np.cos(emb.astype(np.float32)).T.astype(np.float32)  # [D, S]
    sinT = np.sin(emb.astype(np.float32)).T.astype(np.float32)
    sinTpm = np.concatenate([-sinT[:64], sinT[64:]], axis=0)
    return cosT, sinTpm


def _mask_tiles():
    c = np.arange(128)[:, None]
    u = np.arange(128)[None, :]
    diag_keep = (u >= c).astype(np.float32)  # causal diag block
    i2_keep = (u <= c).astype(np.float32)  # window tail block
    return np.stack([diag_keep, i2_keep], axis=1)  # [128, 2, 128]


def _swap_halves(xT):
    return np.concatenate([xT[64:], xT[:64]], axis=0)


def _install_ntff_shim():
    """Provide antenv.axon_hooks (NTFF profile hook) if the image lacks it,
    so run_bass_kernel_spmd(trace=True) can capture HW profiles via the
    axon PJRT .so. Silently no-ops if unavailable."""
    import sys
    import types

    try:
        from antenv.axon_hooks import get_axon_ntff_profile_hook  # noqa: F401

        return
    except ImportError:
        pass
    try:
        import contextlib
        import ctypes

        lib = ctypes.CDLL("/opt/axon/libaxon_pjrt.so")
        if not hasattr(lib, "axon_start_nrt_profile"):
            return
        lib.axon_start_nrt_profile.argtypes = [
            ctypes.POINTER(ctypes.c_int64),
            ctypes.c_size_t,
        ]
        lib.axon_start_nrt_profile.restype = ctypes.c_int64
        lib.axon_stop_nrt_profile.argtypes = [ctypes.c_char_p]
        lib.axon_stop_nrt_profile.restype = ctypes.c_int64

        @contextlib.contextmanager
        def _hook(output_dir, device_ids):
            import jax

            jax.devices()
            if device_ids:
                ids = (ctypes.c_int64 * len(device_ids))(*device_ids)
                rc = lib.axon_start_nrt_profile(ids, len(device_ids))
            else:
                rc = lib.axon_start_nrt_profile(None, 0)
            if rc != 0:
                raise RuntimeError(f"axon_start_nrt_profile rc={rc}")
            try:
                yield
            finally:
                n = lib.axon_stop_nrt_profile(str(output_dir).encode())
                print(f"ntff profile: {n} file(s) -> {output_dir}", file=sys.stderr)

        mod = types.ModuleType("antenv.axon_hooks")
        mod._hook = _hook
        mod.get_axon_ntff_profile_hook = lambda: _hook
        mod.set_axon_ntff_profile_hook = lambda h: setattr(mod, "_hook", h)
        import antenv

        antenv.axon_hooks = mod
        sys.modules["antenv.axon_hooks"] = mod
    except Exception:
        pass


def kernel(query_states, key_states, value_states):
    from concourse.bass_utils import run_bass_kernel_spmd

    _install_ntff_shim()

    nc = _build_program()

    q = np.asarray(query_states)[0]  # [S, 4096]
    k = np.asarray(key_states)[0]  # [S, 1024]
    v = np.asarray(value_states)[0]  # [S, 1024]

    cosT, sinTpm = _rope_tables()
    cs = np.stack([cosT, sinTpm], axis=0).astype(BF16)  # [2, D, S]
    msk = _mask_tiles().astype(BF16)

    in_maps = []
    for c in range(N_CORES):
        qt = np.empty((HQ_PER_CORE, 2, D, S), dtype=BF16)
        for hh in range(HQ_PER_CORE):
            h = 4 * c + hh
            qh = np.ascontiguousarray(q[:, h * D : (h + 1) * D].T)  # [D, S]
            qt[hh, 0] = qh.astype(BF16)
            qt[hh, 1] = _swap_halves(qh).astype(BF16)
        kh = np.ascontiguousarray(k[:, c * D : (c + 1) * D].T)
        kt = np.stack([kh, _swap_halves(kh)], axis=0).astype(BF16)
        vh = v[:, c * D : (c + 1) * D]  # [S, D]
        vts = np.ascontiguousarray(
            vh.reshape(NB, 128, D).transpose(1, 0, 2)
        ).astype(BF16)  # [kv_local, j, dv]
        in_maps.append({"qt": qt, "kt": kt, "v": vts, "cs": cs, "msk": msk})

    res = run_bass_kernel_spmd(nc, in_maps, core_ids=list(range(N_CORES)))
    global LAST_RESULT
    LAST_RESULT = res

    out = np.empty((S, 32, D), dtype=np.float32)
    for c in range(N_CORES):
        o = res.results[c]["out"]  # [4, D, S] f32
        out[:, 4 * c : 4 * c + 4, :] = o.transpose(2, 0, 1)
    return out.reshape(1, S, 32 * D)
